# revision 103
# baseline (speedup 1.0000x reference)
"""Causal self-attention (dense transformer block) on 8 Trainium2 NeuronCores.

Sharding: core c handles batch b = c//2 and head-group g = c%2 (8 of 16 heads).
Single fused pipeline over 512-row sequence slices. Engines execute their
instruction streams strictly in order, so emission order is the schedule: a
greedy weave interleaves QKV projection groups, QK score pairs (spread to the
Act engine's exp pace), PV accumulation quanta, and output-projection groups
(pipelined one slice behind). Scores use the transposed layout (keys on
partitions); V carries a 64-wide ones block so the softmax denominator lands
replicated on partitions 64-127 (reciprocal + multiply, no DMA broadcast).
Each core emits a partial [2048, 1024] output the host sums per batch pair.

All shapes hardcoded for x[4, 2048, 1024], 16 heads, head_dim 64.
"""
import sys

sys.path.insert(0, "/opt/trn_rl_repo")

import contextlib

import ml_dtypes
import numpy as np

import concourse.bass as bass
import concourse.tile as tile
from concourse import mybir
from concourse.bass_utils import run_bass_kernel_spmd
from concourse.masks import make_upper_triangular

F32 = mybir.dt.float32
BF16 = mybir.dt.bfloat16
F8 = mybir.dt.float8e4
EXP = mybir.ActivationFunctionType.Exp

# head-pairs whose QK runs in fp8e4 DoubleRow (half PE cost); per-pair
# error ~1.9e-2 scaled by sqrt(|FP8_PAIRS|/4) stays well under the 2e-2
# gate alongside the ~4e-3 bf16 baseline error
FP8_PAIRS = (0, 1, 2, 3)
BF16_PAIRS = tuple(p for p in range(4) if p not in FP8_PAIRS)

SEQ = 2048
DM = 1024
M = 512          # per-core qkv output dims (8 heads x 64)
HD = 64
NHC = 8          # heads per core
NJT = 16         # 128-row key tiles

PE_NS = 0.4167   # ns per PE cycle at full clock
ACT_NS = 0.833

# qkv weights are scaled by WS on the host so their fp8 hi/lo split stays in
# e4m3's normal range; q,k pick up WS (folded into the exp scale), v picks up
# WS (cancelled by the ones-block denominator, memset to WS)
WS = 64.0
EXP_SCALE = 0.125 / (WS * WS)


def _split_multiwaits(nc, limit=1):
    """walrus in this container rejects >1 sync-wait per instruction; move
    extra waits onto same-engine nops placed directly before."""
    n = 0
    for func in nc.m.functions:
        for blk in func.blocks:
            out = []
            for inst in blk.instructions:
                si = inst.sync_info
                if si is not None and len(si.on_wait) > limit:
                    waits = list(si.on_wait)
                    for w in waits[:-limit]:
                        n += 1
                        out.append(mybir.InstNoOp(
                            name=f"I-waitsplit-{n}", engine=inst.engine,
                            bass_nofuse=True,
                            sync_info=mybir.SyncInfo(on_wait=[w], on_update=[])))
                    inst.sync_info = mybir.SyncInfo(
                        on_wait=waits[-limit:], on_update=list(si.on_update))
                out.append(inst)
            blk.instructions = out
    return n


def _build_nc(repeat=1):
    nc = bass.Bass("TRN2", target_bir_lowering=False, debug=False,
                   enable_asserts=False, num_devices=1)
    xts = [nc.dram_tensor(f"xt{s}", [DM, SEQ], F8,
                          kind="ExternalInput").ap() for s in "hl"]
    ws = {}
    for wname in ("wq", "wk", "wv"):
        for s in "hl":
            ws[wname + s] = nc.dram_tensor(
                wname + s, [DM, M], F8, kind="ExternalInput").ap()
    wp = nc.dram_tensor("wp", [M, DM], BF16, kind="ExternalInput").ap()
    out = nc.dram_tensor("out", [SEQ, DM], BF16, kind="ExternalOutput").ap()

    with tile.TileContext(nc) as tc:
        for rep in range(repeat):
            with contextlib.ExitStack() as ctx:
                _body(nc, tc, ctx, xts, ws, wp, out, rep)
    _split_multiwaits(nc)
    return nc


def _body(nc, tc, ctx, xts, ws, wp, out, rep=0):
    persist = ctx.enter_context(tc.tile_pool(name=f"persist{rep}", bufs=1))
    # k (bf16 pairs only): [dmod-of-2-heads (128), bf16-pair, seq]
    k_sb = persist.tile([128, max(1, len(BF16_PAIRS)), SEQ], BF16, tag="k")
    # fp8 pairs: [32*c2 + hd%32, hd-half, pair, seq] (weights host-permuted)
    k8_sb = persist.tile([64, 2, len(FP8_PAIRS), SEQ], F8, tag="k8")
    # v (natural) + 64 ones cols: [key-partition, key-tile, head, hd+64]
    v_sb = persist.tile([128, NJT, NHC, 128], BF16, tag="v")
    tri = persist.tile([128, 128], BF16, tag="tri")

    qpool = ctx.enter_context(tc.tile_pool(name=f"qp{rep}", bufs=2))
    ytpool = ctx.enter_context(tc.tile_pool(name=f"ytp{rep}", bufs=2))
    wpool = ctx.enter_context(tc.tile_pool(name=f"wt{rep}", bufs=1))
    xpool = ctx.enter_context(tc.tile_pool(name=f"xs{rep}", bufs=2))
    ppool = ctx.enter_context(tc.tile_pool(name=f"psb{rep}", bufs=37))
    mpool = ctx.enter_context(tc.tile_pool(name=f"misc{rep}", bufs=4))
    opool = ctx.enter_context(tc.tile_pool(name=f"osb{rep}", bufs=3))
    # PSUM: 8 banks = scores 2x[128,1024] (4) + qkv/proj work 2 + pv 2
    s_pool = ctx.enter_context(
        tc.tile_pool(name=f"s_ps{rep}", bufs=2, space="PSUM"))
    w_ps = ctx.enter_context(
        tc.tile_pool(name=f"w_ps{rep}", bufs=2, space="PSUM"))
    y_ps_pool = ctx.enter_context(
        tc.tile_pool(name=f"y_ps{rep}", bufs=2, space="PSUM"))

    # ---- input DMAs, ordered so the first matmul's deps land fastest ----
    xt_rs = [x.rearrange("(dt p) s -> p dt s", p=128) for x in xts]
    x_t = [None] * 4

    def load_x(ss):
        pair = []
        for hl in range(2):
            t = xpool.tile([128, 8, 512], F8, tag=f"x{hl}",
                           name=f"x{hl}_{rep}_{ss}")
            # ss=0: a small first hi piece so the first matmuls start
            # sooner; everything else in one DMA per tensor
            pieces = ((0, 2), (2, 8)) if ss == 0 and hl == 0 else ((0, 8),)
            for d0, d1 in pieces:
                nc.sync.dma_start(
                    t[:, d0:d1, :],
                    xt_rs[hl][:, d0:d1, 512 * ss:512 * ss + 512])
            pair.append(t)
        x_t[ss] = pair

    # x(ss=0) leads the sync ring; wq split across Pool+Act SWDGE queues
    # (first tiles land fastest; both queues clear before their real work)
    w_tiles = {}
    load_x(0)
    # one DMA per weight tensor (per-DMA overhead dominates these small
    # fp8 transfers); DoubleRow pair-slices address subranges of the tile
    def wload(wname, hl, eng):
        wr = ws[wname + hl].rearrange("(dp p) m -> p dp m", p=128)
        t = wpool.tile([128, 8, M], F8, tag=f"{wname}{hl}",
                       name=f"{wname}{hl}{rep}")
        eng.dma_start(t[:], wr[:])
        for dp in range(4):
            w_tiles[(wname, hl, dp)] = t[:, 2 * dp:2 * dp + 2, :]

    wload("wq", "h", nc.gpsimd)
    wload("wq", "l", nc.scalar)
    wload("wk", "h", nc.gpsimd)
    wload("wk", "l", nc.scalar)
    wload("wv", "h", nc.sync)
    wload("wv", "l", nc.sync)
    # warm the Act exp table while its queue is otherwise clear
    warm = mpool.tile([1, 2], F32, tag="warm")
    nc.vector.memset(warm[:], 0.0)
    nc.scalar.activation(warm[:], warm[:], EXP)
    wp_sb = []
    for dt in range(4):
        wt = wpool.tile([128, DM], BF16, tag=f"wp{dt}")
        nc.sync.dma_start(wt[:], wp[128 * dt:128 * dt + 128, :])
        wp_sb.append(wt)

    # constants: tri mask + the ones block of v (jt 0-3 first: PV(ti=0)
    # needs them earliest)
    make_upper_triangular(nc, tri[:], val=1.0, diag=True)
    for jt in range(NJT):
        nc.gpsimd.memset(v_sb[:, jt, :, HD:128], WS)

    p_tiles = {}
    q_cur = [None]
    y_cur = [None]
    y_ps = {}

    # ---------------- quantum emitters ----------------
    def emit_qk_group(ss, wname, mt, q_t, q8_t):
        ps = w_ps.tile([128, 512], F32, tag="w",
                       name=f"g_{rep}_{ss}_{wname}_{mt}")
        xh, xl = x_t[ss]
        steps = [("h", xh), ("l", xh), ("h", xl)]
        for i, (whl, xt_t) in enumerate(steps):
            for dp in range(4):
                nc.tensor.matmul(
                    ps[:],
                    w_tiles[(wname, whl, dp)][:, :, 128 * mt:128 * mt + 128],
                    xt_t[:, 2 * dp:2 * dp + 2, :],
                    start=(i == 0 and dp == 0), stop=(i == 2 and dp == 3),
                    perf_mode=mybir.MatmulPerfMode.DoubleRow,
                    skip_group_check=True)
        if mt in FP8_PAIRS:
            pr = FP8_PAIRS.index(mt)
            for i in range(2):
                if wname == "wq":
                    nc.vector.tensor_copy(q8_t[:, i, pr, :],
                                          ps[64 * i:64 * i + 64, :])
                else:
                    nc.vector.tensor_copy(
                        k8_sb[:, i, pr, 512 * ss:512 * ss + 512],
                        ps[64 * i:64 * i + 64, :])
        elif wname == "wq":
            nc.vector.tensor_copy(q_t[:, BF16_PAIRS.index(mt), :], ps[:])
        else:
            nc.vector.tensor_copy(
                k_sb[:, BF16_PAIRS.index(mt), 512 * ss:512 * ss + 512], ps[:])

    def emit_v_group(ss, st):
        ps = w_ps.tile([128, 512], F32, tag="w", name=f"g_{rep}_{ss}_v_{st}")
        xh, xl = x_t[ss]
        steps = [(xh, "h"), (xh, "l"), (xl, "h")]
        for i, (xt_t, whl) in enumerate(steps):
            for dp in range(4):
                nc.tensor.matmul(
                    ps[:],
                    xt_t[:, 2 * dp:2 * dp + 2, 128 * st:128 * st + 128],
                    w_tiles[("wv", whl, dp)],
                    start=(i == 0 and dp == 0), stop=(i == 2 and dp == 3),
                    perf_mode=mybir.MatmulPerfMode.DoubleRow,
                    skip_group_check=True)
        jt = 4 * ss + st
        nc.vector.tensor_copy(
            v_sb[:, jt, :, 0:HD], ps[:].rearrange("p (h d) -> p h d", h=NHC))

    def emit_scores(ti, hp, jt, q_t, q8_t):
        off = 128 * (jt - 4 * ti) if jt >= 4 * ti else 0
        s_ps = s_pool.tile([128, 1024], F32, tag="s",
                           name=f"s_{rep}_{ti}_{hp}_{jt}")
        for c2 in range(2):
            lo = 64 * c2
            if hp in FP8_PAIRS:
                pr = FP8_PAIRS.index(hp)
                nc.tensor.matmul(
                    s_ps[:, 512 * c2 + off:512 * c2 + 512],
                    k8_sb[32 * c2:32 * c2 + 32, :, pr,
                          128 * jt:128 * jt + 128],
                    q8_t[32 * c2:32 * c2 + 32, :, pr, off:512],
                    start=True, stop=True,
                    perf_mode=mybir.MatmulPerfMode.DoubleRow)
            else:
                bi = BF16_PAIRS.index(hp)
                nc.tensor.matmul(
                    s_ps[:, 512 * c2 + off:512 * c2 + 512],
                    k_sb[lo:lo + 64, bi, 128 * jt:128 * jt + 128],
                    q_t[lo:lo + 64, bi, off:512],
                    start=True, stop=True)
        p_t = ppool.tile([128, 1024], BF16, tag="p",
                         name=f"p_{rep}_{ti}_{hp}_{jt}")
        nc.scalar.activation(
            p_t.rearrange("p (c q) -> p c q", c=2)[:, :, off:512],
            s_ps.rearrange("p (c q) -> p c q", c=2)[:, :, off:512],
            EXP, scale=EXP_SCALE)
        if jt >= 4 * ti:
            # on Pool: keeps the DVE queue clear for the recip/mul drains
            for c2 in range(2):
                band = p_t[:, 512 * c2 + off:512 * c2 + off + 128]
                nc.gpsimd.tensor_mul(band, band, tri[:])
        p_tiles[(ti, hp, jt)] = p_t

    def emit_pv(ti, hp, c2, jt, y_t):
        n_j = 4 * ti + 4
        off = 128 * (jt - 4 * ti) if jt >= 4 * ti else 0
        if jt == 0:
            y_ps[(ti, hp, c2)] = y_ps_pool.tile(
                [128, 512], F32, tag="y", name=f"y_{rep}_{ti}_{hp}_{c2}")
        h = 2 * hp + c2
        nc.tensor.matmul(
            y_ps[(ti, hp, c2)][:, off:512],
            v_sb[:, jt, h, :],
            p_tiles[(ti, hp, jt)][:, 512 * c2 + off:512 * c2 + 512],
            start=(jt == 0), stop=(jt == n_j - 1),
            skip_group_check=True)
        if jt == n_j - 1:
            lo = 64 * c2
            rc = mpool.tile([64, 512], F32, tag="rc")
            nc.vector.reciprocal(rc[:], y_ps[(ti, hp, c2)][64:128, :])
            if ti == 3:
                # per-st chunks: the trailing projection unblocks sooner
                for st in range(4):
                    nc.vector.tensor_mul(
                        y_t[lo:lo + 64, hp, 128 * st:128 * st + 128],
                        y_ps[(ti, hp, c2)][0:64, 128 * st:128 * st + 128],
                        rc[:, 128 * st:128 * st + 128])
            else:
                nc.vector.tensor_mul(
                    y_t[lo:lo + 64, hp, :], y_ps[(ti, hp, c2)][0:64, :],
                    rc[:])

    def make_proj(ss, y_t):
        ots = {}

        def emit(st, e):
            s0 = 512 * ss + 128 * st
            last = ss == 3 and st == 3
            # the very last group runs in 256-col halves so its copies and
            # stores start sooner, shortening the end-of-kernel drain
            for half in range(2 if (last and e == 1) else 1):
                cols = 256 if (last and e == 1) else 512
                c0 = 512 * e + 256 * half
                op = w_ps.tile([128, 512], F32, tag="w",
                               name=f"o_{rep}_{ss}_{st}_{e}_{half}")
                # accumulate in the slice's mul-completion order so the
                # group starts before the last pair's normalization lands
                dts = [2, 3, 0, 1] if ss == 3 else [0, 1, 2, 3]
                for i, dt in enumerate(dts):
                    nc.tensor.matmul(
                        op[0:128, 0:cols], y_t[:, dt, 128 * st:128 * st + 128],
                        wp_sb[dt][:, c0:c0 + cols],
                        start=(i == 0), stop=(i == 3))
                if e == 0 and half == 0:
                    ots[st] = opool.tile([128, 1024], BF16, tag="ot",
                                         name=f"ot_{rep}_{ss}_{st}")
                if ss == 3:
                    # tail: Act is idle once the exps drain; keep DVE free
                    # for the last recip/mul chain
                    nc.scalar.activation(ots[st][:, c0:c0 + cols],
                                         op[0:128, 0:cols],
                                         mybir.ActivationFunctionType.Copy)
                else:
                    nc.vector.tensor_copy(ots[st][:, c0:c0 + cols],
                                          op[0:128, 0:cols])
                if last:
                    (nc.gpsimd if (e, half) == (1, 0) else nc.sync).dma_start(
                        out[s0:s0 + 128, c0:c0 + cols],
                        ots[st][:, c0:c0 + cols])
                elif e == 1:
                    (nc.gpsimd if st % 2 == 0 else nc.sync).dma_start(
                        out[s0:s0 + 128, :], ots[st][:])
        return [(lambda st=st, e=e: emit(st, e)) for st in range(4)
                for e in range(2)]

    # ---------------- global greedy weave over all slices ----------------
    class Slice:
        pass

    slices = []
    for ss in range(4):
        sl = Slice()
        sl.ss = sl.ti = ss
        sl.n_j = 4 * ss + 4
        sl.started = False
        sl.q_t = sl.y_t = None
        # fp8 pairs first at the last slice: the tail's remaining PE work
        # per queued exp is larger for bf16 pairs, hiding the exp pacing
        hp_order = [2, 3, 0, 1] if ss == 3 else [0, 1, 2, 3]
        sl.heavies = []
        for i, mt in enumerate(hp_order):
            sl.heavies.append(("qk", "wq", mt))
            sl.heavies.append(("qk", "wk", mt))
            if i >= 1:
                sl.heavies.append(("v", i - 1))
        sl.heavies.append(("v", 3))
        sl.scores = [(hp, jt) for hp in hp_order for jt in range(sl.n_j)]
        sl.pvs = [(hp, c2, jt) for hp in hp_order for c2 in range(2)
                  for jt in range(sl.n_j)]
        sl.projs = None        # created once all pvs are emitted
        sl.scores_emitted = set()
        sl.qk_done = [False] * 4
        sl.v_done = [False] * 4
        slices.append(sl)

    pe_t = 0.0
    act_t = 0.0
    exp_end = []
    exp_by_key = {}
    last_heavy = False

    def start_slice(sl):
        sl.started = True
        if sl.ss < 3:
            load_x(sl.ss + 1)
        sl.q_t = qpool.tile([128, max(1, len(BF16_PAIRS)), 512], BF16,
                            tag="q", name=f"q{rep}_{sl.ss}")
        sl.q8_t = qpool.tile([64, 2, len(FP8_PAIRS), 512], F8, tag="q8",
                             name=f"q8{rep}_{sl.ss}")
        sl.y_t = ytpool.tile([128, 4, 512], BF16, tag="y",
                             name=f"y{rep}_{sl.ss}")

    def do_score(sl):
        nonlocal pe_t, act_t
        hp, jt = sl.scores.pop(0)
        off = 128 * (jt - 4 * sl.ti) if jt >= 4 * sl.ti else 0
        emit_scores(sl.ti, hp, jt, sl.q_t, sl.q8_t)
        sl.scores_emitted.add((hp, jt))
        mul = 1 if hp in FP8_PAIRS else 2
        pe_t += mul * (512 - off) * PE_NS / 1000
        table = 1.28 if not exp_end else 0.0
        act_t = max(act_t, pe_t) + table + \
            ((1024 - 2 * off) * ACT_NS + 143) / 1000
        exp_end.append(act_t)
        exp_by_key[(sl.ti, hp, jt)] = act_t

    def do_pv(sl):
        nonlocal pe_t
        hp, c2, jt = sl.pvs.pop(0)
        off = 128 * (jt - 4 * sl.ti) if jt >= 4 * sl.ti else 0
        emit_pv(sl.ti, hp, c2, jt, sl.y_t)
        pe_t += (512 - off) * PE_NS / 1000
        if not sl.pvs:
            sl.projs = make_proj(sl.ss, sl.y_t)

    def do_heavy(sl):
        nonlocal pe_t
        if not sl.started:
            start_slice(sl)
        h = sl.heavies.pop(0)
        if h[0] == "qk":
            _, wname, mt = h
            emit_qk_group(sl.ss, wname, mt, sl.q_t, sl.q8_t)
            if wname == "wk":
                sl.qk_done[mt] = True
        else:
            _, st = h
            emit_v_group(sl.ss, st)
            sl.v_done[st] = True
        pe_t += 6 * 512 * PE_NS / 1000

    def do_proj(sl):
        nonlocal pe_t
        sl.projs.pop(0)()
        pe_t += 4 * 512 * PE_NS / 1000

    def score_ok(sl):
        if not sl.scores or not sl.started:
            return False
        hp, jt = sl.scores[0]
        return sl.qk_done[hp]

    def pv_ok(sl):
        if not sl.pvs:
            return False
        hp, c2, jt = sl.pvs[0]
        if (hp, jt) not in sl.scores_emitted:
            return False
        if jt >= 4 * sl.ti and not sl.v_done[jt - 4 * sl.ti]:
            return False
        return True

    def pv_exp_done(sl):
        hp, c2, jt = sl.pvs[0]
        return exp_by_key.get((sl.ti, hp, jt), 1e9) <= pe_t + 0.2

    while True:
        live = [sl for sl in slices
                if sl.heavies or sl.scores or sl.pvs or sl.projs]
        if not live:
            break
        oldest = live[0]
        # cap run-ahead at one slice past the oldest incomplete one
        horizon = [sl for sl in live if sl.ss <= oldest.ss + 1]
        did = False
        # 1. oldest slice's scores at the exp pace (s-pool rotation)
        for sl in horizon:
            slack = 3.0 if sl.ti == 0 else 0.6
            if score_ok(sl) and (len(exp_end) < 2
                                 or exp_end[-2] <= pe_t + slack):
                do_score(sl)
                last_heavy = False
                did = True
                break
        if did:
            continue
        # 2. Act about to starve and no score emittable: pull forward the
        # heavies that unlock the next slice's scores
        if act_t <= pe_t + 1.5:
            hv = next((sl for sl in horizon if sl.heavies and sl.scores),
                      None)
            if hv is not None:
                do_heavy(hv)
                last_heavy = True
                continue
        # 3. pvs whose exp is already done; at the small early slices the
        # chains are shorter than their recip/mul drain, so chase each pv
        # with a heavy to hide the y-bank turnaround
        if pv_ok(oldest) and pv_exp_done(oldest):
            do_pv(oldest)
            if oldest.ti <= 1:
                hv = next((sl for sl in horizon if sl.heavies), None)
                if hv is not None:
                    do_heavy(hv)
            last_heavy = False
            continue
        # 3. heavies (alternating with proj groups as w_ps spacers)
        hv = next((sl for sl in horizon if sl.heavies), None)
        pj = next((sl for sl in horizon if sl.projs), None)
        if hv is not None and (not last_heavy or pj is None):
            do_heavy(hv)
            last_heavy = True
            continue
        if pj is not None:
            do_proj(pj)
            last_heavy = False
            continue
        # 4. forced (pipeline-stalling) fallbacks, oldest first
        if pv_ok(oldest):
            do_pv(oldest)
            last_heavy = False
            continue
        forced = next((sl for sl in horizon if score_ok(sl)), None)
        if forced is not None:
            do_score(forced)
            last_heavy = False
            continue
        raise AssertionError("weave deadlock")


_NC = None


def _get_nc():
    global _NC
    if _NC is None:
        _NC = _build_nc()
    return _NC


def _permute_fp8_groups(w):
    """w: [1024, 512]. For fp8 head-pairs, reorder each 128-col group's
    32-col blocks [A0|A32|B0|B32] -> [A0|B0|A32|B32] so on-device q/k
    PSUM partitions split into DoubleRow hd-halves with plain copies."""
    w = np.ascontiguousarray(w).reshape(1024, 4, 4, 32)
    w[:, list(FP8_PAIRS)] = w[:, list(FP8_PAIRS)][:, :, [0, 2, 1, 3]]
    return w.reshape(1024, 512)


def _split8(a):
    hi = a.astype(ml_dtypes.float8_e4m3fn)
    lo = (a - hi.astype(np.float32)).astype(ml_dtypes.float8_e4m3fn)
    return np.ascontiguousarray(hi), np.ascontiguousarray(lo)


def _core_inputs(x, w_qkv, w_proj, core):
    b, g = core // 2, core % 2
    ms = slice(512 * g, 512 * g + 512)
    xh, xl = _split8(x[b].T)
    wqh, wql = _split8(_permute_fp8_groups(w_qkv[0:1024][ms].T * WS))
    wkh, wkl = _split8(_permute_fp8_groups(w_qkv[1024:2048][ms].T * WS))
    wvh, wvl = _split8(w_qkv[2048:3072][ms].T * WS)
    return {
        "xth": xh, "xtl": xl, "wqh": wqh, "wql": wql,
        "wkh": wkh, "wkl": wkl, "wvh": wvh, "wvl": wvl,
        "wp": np.ascontiguousarray(w_proj[:, ms].T.astype(ml_dtypes.bfloat16)),
    }


def kernel(x, w_qkv, w_proj):
    x = np.asarray(x, dtype=np.float32)
    w_qkv = np.asarray(w_qkv, dtype=np.float32)
    w_proj = np.asarray(w_proj, dtype=np.float32)
    nc = _get_nc()
    in_maps = [_core_inputs(x, w_qkv, w_proj, c) for c in range(8)]
    res = run_bass_kernel_spmd(nc, in_maps, core_ids=list(range(8)))
    out = np.empty((4, SEQ, DM), dtype=np.float32)
    for b in range(4):
        out[b] = (res.results[2 * b]["out"].astype(np.float32)
                  + res.results[2 * b + 1]["out"].astype(np.float32))
    return out


if __name__ == "__main__":
    rng = np.random.default_rng(0)
    x = rng.standard_normal((4, SEQ, DM), dtype=np.float32)
    w_qkv = (rng.random((3 * DM, DM), dtype=np.float32) - 0.5) / 16.0
    w_proj = (rng.random((DM, DM), dtype=np.float32) - 0.5) / 16.0
    y = kernel(x, w_qkv, w_proj)
    print("ok", y.shape, float(np.abs(y).mean()))
